# revision 2
# baseline (speedup 1.0000x reference)
"""Trainium2 Bass kernel for nn_DiagonalMicroAttention (3x3 neighborhood sparse attention).

Final: PE p-state warmup (junk matmuls bridging the input DMA), priority-ordered
emission for the list scheduler, batch-major pp products on DVE, tanh-based
sigmoid (gelu/tanh + exp LUT sets only, preloaded off the critical path),
asym resize algebra folded to 2 STT + 1 TS on Pool, bf16 softmax tail,
3-way K-evac, rob muls spread DVE-direct/Act-evac/Pool-direct, manually
sequenced PSUM pools so dots/ebp/out banks open as phase-A banks retire.

Sharding: 8 cores x 7 query rows (both batches per core). Channel-major layout.
"""
import numpy as np

import concourse.bass as bass
from concourse.ap import AP
import concourse.tile as tile
from concourse import bacc, mybir
from concourse.bass_utils import run_bass_kernel_spmd

F32 = mybir.dt.float32
BF16 = mybir.dt.bfloat16
AF = mybir.ActivationFunctionType
ALU = mybir.AluOpType

B, C, H, W, HEADS = 2, 128, 56, 56, 4
DH = C // HEADS
SCALE = float(DH) ** -0.5
NCORES = 8
RQ, RH, WP = 7, 9, 58
NQ = RQ * W          # 392 queries per batch per core
BIG = 30000.0
S = 100
S36 = 36
NJUNK = 7

# pkb1 (bf16, 128 x 2240): inputs + projection weights
XS, XF, WQ, WK, WV, WA, WB, W2 = 0, 1044, 1436, 1564, 1692, 1820, 1948, 2076
NPKB1 = 2240
# pkb2 (bf16, 128 x 2044): base3, wo, esel9, mask, den selector
B3, WOB, E9, MT, MV, DENB = 0, 172, 300, 1452, 1552, 1944
NPKB2 = 2044
DEN = 3
BA2, BA1, BO = 0, 1, 2
NPKF = 103

# rob engine per offset o: d=DVE-direct, a=Act-evac+DVE-mul, p=Pool-direct
import os as _os
ROB_ENG = [list(_os.environ.get("ROB0", "dqadqadqa")),
           list(_os.environ.get("ROB1", "dqadqadqa"))]
# pp ops offloaded to Pool: "b0:12,b1:0123" style
_ppp = _os.environ.get("PP_POOL", "")
PP_POOL = set()
for part in _ppp.split(","):
    if ":" in part:
        bs, os_ = part.split(":")
        for ch in os_:
            PP_POOL.add((int(bs[1]), int(ch)))
O_ORDER = list(range(9))


def _emit_body(nc, tc, v):
    work, robpool = v["work"], v["robpool"]
    pkb1, pkb2, pkf, out_d = v["pkb1"], v["pkb2"], v["pkf"], v["out_d"]

    qsb = work.tile([C, B, RQ, W], BF16, tag="qsb")
    ksb = work.tile([C, B, RH, WP], BF16, tag="ksb")
    vsb = work.tile([C, B, RH, WP], BF16, tag="vsb")
    pp = work.tile([C, B, 9, RQ, W], BF16, tag="pp")
    first = "t1_0" not in v
    if first:
        v["a1s"] = work.tile([C, B, RQ, 28], BF16, tag="a1s", name="a1s")
        for b in range(B):
            v[f"apad_{b}"] = work.tile([S36, RQ, 30], BF16, tag=f"apad{b}",
                                       name=f"apad_{b}")
            v[f"u_{b}"] = work.tile([S36, RQ, 28, 2], BF16, tag=f"u{b}",
                                    name=f"u_{b}")
            v[f"t1_{b}"] = work.tile([S36, NQ], BF16, tag=f"t1{b}",
                                     name=f"t1_{b}")
        v["dummy"] = work.tile([1, 1], F32, tag="dummy", name="dummy")
    a1s, dummy = v["a1s"], v["dummy"]
    osb = work.tile([C, B, RQ, W], F32, tag="osb")
    e_t = [work.tile([S36, NQ], BF16, tag=f"e{b}", name=f"e_{b}")
           for b in range(B)]
    r_t = [work.tile([S36, NQ], F32, tag=f"r{b}", name=f"r_{b}")
           for b in range(B)]
    et1 = [work.tile([S36, NQ], BF16, tag=f"et1{b}", name=f"et1_{b}")
           for b in range(B)]
    e2 = [work.tile([S36, NQ], BF16, tag=f"e2{b}", name=f"e2_{b}")
          for b in range(B)]

    xs4 = pkb1[:, XS:XS + 1044].rearrange("p (b r c) -> p b r c", b=B, r=RH)
    xsf = pkb1[:, XS:XS + 1044]
    xf4 = pkb1[:, XF:XF + 392].rearrange("p (b r c) -> p b r c", b=B, r=RQ)
    wob = pkb2[:, WOB:WOB + C]
    ba1_v = pkf[:, BA1:BA1 + 1]
    bo_v = pkf[:, BO:BO + 1]
    ba2h_v = pkf[0:S36, BA2:BA2 + 1]   # host stores ba2*0.5
    mt_v = pkb2[0:4, MT:MT + S36]
    mv_v = pkb2[0:4, MV:MV + NQ]
    w2v = pkb1[:, W2 + 64:W2 + 64 + S36]

    def kshift(t, o):
        di, dj = o // 3 - 1, o % 3 - 1
        return t[:, :, 1 + di:8 + di, 1 + dj:57 + dj]

    NKV = B * RH * WP  # 1044

    # ---- single PSUM pool; bank recycling via per-tag rotation ----
    # tag bankA (2 bufs): qps_b -> dots_b -> den_b
    # tag bankB (4 bufs): K chunks -> V chunks -> ebp tiles
    # tag bankC (2 bufs): a1ps -> a2ps_b -> outp_b
    cmps = tc.tile_pool(name="ps", bufs=1, space="PSUM")
    ps = cmps.__enter__()

    qps = [ps.tile([C, 512], F32, tag="bankA", bufs=2, name=f"qps_{b}")
           for b in range(B)]
    kv = [ps.tile([C, 512], F32, tag="bankB", bufs=4, name=f"kv_{j}")
          for j in range(2)]
    kv.append(ps.tile([C, 20], F32, tag="bankB", bufs=4, name="kv_2"))
    a1ps = ps.tile([C, 392], F32, tag="bankC", bufs=2, name="a1ps")

    if first:
        jw = v["junkw"]
        for i in range(6):
            nc.tensor.matmul(qps[0][0:16, :], jw[:, 0:16], jw[:, :],
                             start=True, stop=True)
        for i in range(3):
            nc.tensor.matmul(qps[0][0:16, 0:128], jw[:, 0:16], jw[:, 0:128],
                             start=True, stop=True)
    # Q projection
    for b in range(B):
        nc.tensor.matmul(qps[b][:, 0:NQ], pkb1[:, WQ:WQ + C],
                         xs4[:, b, 1:8, 1:57], start=True, stop=True)
    nc.scalar.copy(out=qsb[:, 0, :, :], in_=qps[0][:, 0:NQ])
    nc.vector.tensor_copy(out=qsb[:, 1, :, :], in_=qps[1][:, 0:NQ])
    # asym a1 early so gelu->a2->tanh->exp-LUT-load clears Act by ~8us
    skip_asym = v.get("skip_asym")
    if not skip_asym:
        nc.tensor.matmul(a1ps[:, :], pkb1[:, WA:WA + C],
                         xs4[:, :, 1:8, 1:29], start=True, stop=False)
        nc.tensor.matmul(a1ps[:, :], pkb1[:, WB:WB + C],
                         xf4[:, :, :, :], start=False, stop=True)
        nc.scalar.activation(a1s[:, :, :, :], a1ps[:, :], AF.Gelu,
                             bias=ba1_v, scale=1.0)
    # K projection; evacs on DVE so Act is free for the asym chain
    ksf = ksb[:, :, :, :].rearrange("p b r c -> p (b r c)")
    vsf = vsb[:, :, :, :].rearrange("p b r c -> p (b r c)")
    for j, (lo, hi) in enumerate(((0, 512), (512, 1024), (1024, NKV))):
        nc.tensor.matmul(kv[j][:, 0:hi - lo], pkb1[:, WK:WK + C],
                         xsf[:, lo:hi], start=True, stop=True)
    nc.scalar.copy(out=ksf[:, 0:512], in_=kv[0][:, :])
    nc.vector.tensor_copy(out=ksf[:, 512:1024], in_=kv[1][:, :])
    nc.vector.tensor_copy(out=ksf[:, 1024:NKV], in_=kv[2][:, :])
    # V projection (rotates through the same bankB buffers)
    vv_t = [ps.tile([C, 512], F32, tag="bankB", bufs=4, name=f"vv_{j}")
            for j in range(2)]
    vv_t.append(ps.tile([C, 20], F32, tag="bankB", bufs=4, name="vv_2"))
    for j, (lo, hi) in enumerate(((0, 512), (512, 1024), (1024, NKV))):
        nc.tensor.matmul(vv_t[j][:, 0:hi - lo], pkb1[:, WV:WV + C],
                         xsf[:, lo:hi], start=True, stop=True)
    # asym branch (a1/gelu hoisted above, before K)
    if not skip_asym:
        for b in range(B):
            a2ps = ps.tile([S36, RQ * 28], F32, tag="bankC", bufs=2,
                           name=f"a2ps_{b}")
            nc.tensor.matmul(a2ps[:, :], w2v, a1s[:, b, :, :],
                             start=True, stop=True)
            # sigmoid(x) = 0.5 + 0.5*tanh(x/2); the affine is folded into
            # the resize: t1 = (3*tau1 + tau0)/16 + 1.25
            nc.scalar.activation(v[f"apad_{b}"][:, :, 1:29], a2ps[:, :],
                                 AF.Tanh, bias=ba2h_v, scale=0.5)
        # prefetch the Exp LUT set right after the tanhs (ACT idles here)
        v["dummy_inst"] = nc.scalar.activation(
            dummy[:, :], v["apad_1"][0:1, 0:1, 1:2], AF.Exp, scale=1.0)
        for b in range(B):
            apad, u_t = v[f"apad_{b}"], v[f"u_{b}"]
            nc.gpsimd.tensor_copy(out=apad[:, :, 0:1], in_=apad[:, :, 1:2])
            nc.gpsimd.tensor_copy(out=apad[:, :, 29:30],
                                  in_=apad[:, :, 28:29])
            nc.vector.scalar_tensor_tensor(
                out=u_t[:, :, :, 0], in0=apad[:, :, 1:29], scalar=3.0,
                in1=apad[:, :, 0:28], op0=ALU.mult, op1=ALU.add)
            nc.vector.scalar_tensor_tensor(
                out=u_t[:, :, :, 1], in0=apad[:, :, 1:29], scalar=3.0,
                in1=apad[:, :, 2:30], op0=ALU.mult, op1=ALU.add)


    def emit_t1(b):
        if v.get("skip_asym"):
            return
        nc.vector.tensor_scalar(
            out=v[f"t1_{b}"][:, :],
            in0=v[f"u_{b}"][:, :, :, :].rearrange("p r c t -> p (r c t)"),
            scalar1=0.0625, scalar2=1.25, op0=ALU.mult, op1=ALU.add)

    def emit_pp(b):
        for o in O_ORDER:
            if (b, o) in PP_POOL:
                nc.gpsimd.tensor_mul(pp[:, b, o, :, :], qsb[:, b, :, :],
                                     kshift(ksb, o)[:, b])
            else:
                nc.vector.tensor_mul(pp[:, b, o, :, :], qsb[:, b, :, :],
                                     kshift(ksb, o)[:, b])

    dots, den = {}, {}

    def emit_dots(b):
        dots[b] = ps.tile([S36, NQ], F32, tag="bankA", bufs=2, name=f"dots_{b}")
        for i, o in enumerate(O_ORDER):
            s0 = B3 + 72 - o
            nc.tensor.matmul(dots[b][:, :], pkb2[:, s0:s0 + S36],
                             pp[:, b, o, :, :], start=(i == 0), stop=False)
        nc.tensor.matmul(dots[b][:, :], mt_v, mv_v, start=False, stop=True)
        nc.scalar.activation(e_t[b][:, :], dots[b][:, :], AF.Exp, scale=SCALE)

    def emit_tail(b):
        den[b] = ps.tile([S36, NQ], F32, tag="bankA", bufs=2, name=f"den_{b}")
        nc.tensor.matmul(den[b][:, :], pkb2[0:S36, DENB:DENB + S36],
                         e_t[b][:, :], start=True, stop=True)
        nc.vector.reciprocal_approx_fast(r_t[b][:, :], den[b][:, :])
        nc.vector.tensor_mul(et1[b][:, :], e_t[b][:, :], v[f"t1_{b}"][:, :])
        nc.vector.tensor_mul(e2[b][:, :], et1[b][:, :], r_t[b][:, :])

    emit_pp(0)
    emit_t1(0)
    emit_dots(0)
    emit_tail(0)
    nc.scalar.copy(out=vsf[:, 0:512], in_=vv_t[0][:, :])
    nc.scalar.copy(out=vsf[:, 512:1024], in_=vv_t[1][:, :])
    nc.scalar.copy(out=vsf[:, 1024:NKV], in_=vv_t[2][:, :])
    emit_pp(1)
    emit_t1(1)
    emit_dots(1)
    emit_tail(1)

    outp = [ps.tile([C, 512], F32, tag="bankC", bufs=2, name=f"outp_{b}")
            for b in range(B)]

    ebps, robs = {}, {}

    def emit_ebp(b, o):
        ebps[(b, o)] = ps.tile([C, 512], F32, tag="bankB", bufs=4,
                               name=f"ebp_{b}_{o}")
        nc.tensor.matmul(ebps[(b, o)][:, 0:NQ],
                         pkb2[0:S36, E9 + C * o:E9 + C * (o + 1)],
                         e2[b][:, :], start=True, stop=True)

    def emit_rob(b, o):
        vv = kshift(vsb, o)[:, b]
        rob = robpool.tile([C, RQ, W], BF16, tag="rob", name=f"rob_{b}_{o}")
        eng = ROB_ENG[b][o]
        if eng == 'd':
            nc.vector.tensor_mul(rob[:, :, :], ebps[(b, o)][:, 0:NQ], vv)
        else:
            ebsb = work.tile([C, NQ], BF16, tag=f"ebsb{b}_{o % 3}",
                             name=f"ebsb_{b}_{o}")
            nc.scalar.copy(out=ebsb[:, :], in_=ebps[(b, o)][:, 0:NQ])
            if eng == 'q':
                nc.gpsimd.tensor_mul(rob[:, :, :], ebsb[:, :], vv)
            else:
                nc.vector.tensor_mul(rob[:, :, :], ebsb[:, :], vv)
        robs[(b, o)] = rob

    def emit_c(b, split=False):
        for o in range(9):
            emit_ebp(b, o)
            emit_rob(b, o)
        osbf = osb[:, b, :, :].rearrange("p r c -> p (r c)")
        outf = out_d[:, b, :, :].rearrange("p r c -> p (r c)")
        halves = ((0, 196), (196, NQ)) if split else ((0, NQ),)
        for lo, hi in halves:
            for i in range(9):
                nc.tensor.matmul(
                    outp[b][:, lo:hi], wob,
                    robs[(b, i)][:, :, :].rearrange("p r c -> p (r c)")[:, lo:hi],
                    start=(i == 0), stop=(i == 8))
            nc.scalar.activation(osbf[:, lo:hi], outp[b][:, lo:hi],
                                 AF.Identity, bias=bo_v, scale=1.0)
            nc.sync.dma_start(out=outf[:, lo:hi], in_=osbf[:, lo:hi])

    emit_c(0)
    emit_c(1)

    cmps.__exit__(None, None, None)

def build(repeat=1):
    nc = bacc.Bacc(num_devices=NCORES, debug=False)
    pkb1_d = nc.dram_tensor("pkb1", (C, NPKB1), BF16, kind="ExternalInput")
    pkb2_d = nc.dram_tensor("pkb2", (C, NPKB2), BF16, kind="ExternalInput")
    pkf_d = nc.dram_tensor("pkf", (C, NPKF), F32, kind="ExternalInput")
    out_d = nc.dram_tensor("out", (C, B, RQ, W), F32, kind="ExternalOutput")

    with tile.TileContext(nc) as tc:
        with tc.tile_pool(name="consts", bufs=1) as consts, \
             tc.tile_pool(name="work", bufs=1) as work, \
             tc.tile_pool(name="robp", bufs=10) as robpool:
            junkw = consts.tile([C, 512], BF16, tag="junkw")
            dummyg = consts.tile([1, 1], F32, tag="dummyg")
            nc.gpsimd.memset(junkw[:, :], 0.0)
            # preload the Gelu/Tanh LUT set while everything is idle
            nc.scalar.activation(dummyg[:, :], junkw[0:1, 0:1], AF.Gelu,
                                 bias=0.0, scale=1.0)
            pkb1_t = consts.tile([C, NPKB1], BF16, tag="pkb1")
            pkb2_t = consts.tile([C, NPKB2], BF16, tag="pkb2")
            pkf_t = consts.tile([C, NPKF], F32, tag="pkf")
            # x + Wq/Wk/Wv first (gates everything), then pkb2 (dots
            # selectors), then pkf + asym weights via SWDGE
            nc.sync.dma_start(out=pkb1_t[:, 0:WA], in_=pkb1_d[:, 0:WA])
            nc.scalar.dma_start(out=pkb1_t[:, WA:NPKB1],
                                in_=pkb1_d[:, WA:NPKB1])
            nc.scalar.dma_start(out=pkf_t[:, :], in_=pkf_d[:, :])
            nc.scalar.dma_start(out=pkb2_t[:, :], in_=pkb2_d[:, :])
            v = {"pkb1": pkb1_t, "pkb2": pkb2_t, "pkf": pkf_t, "out_d": out_d,
                 "work": work, "robpool": robpool, "junkw": junkw}
            for i in range(repeat):
                v["skip_asym"] = (i > 0)
                _emit_body(nc, tc, v)
    nc.compile()
    return nc


def host_inputs(x, Wq, Wk, Wv, Wo, bo, Wa1, ba1, Wa2, ba2):
    import ml_dtypes
    BF = ml_dtypes.bfloat16
    pkb1 = np.zeros((C, NPKB1), BF)
    pkb1[:, WQ:WQ + C] = Wq.T
    pkb1[:, WK:WK + C] = Wk.T
    pkb1[:, WV:WV + C] = Wv.T
    pkb1[:, WA:WA + C] = Wa1[:, :C].T
    pkb1[:, WB:WB + C] = Wa1[:, C:].T
    w2 = np.zeros((C, 164), np.float32)
    w2[:, 64:100] = Wa2[0][:, None]
    pkb1[:, W2:W2 + 164] = w2

    pkb2 = np.zeros((C, NPKB2), BF)
    base3 = np.zeros((C, 172), np.float32)
    for h in range(HEADS):
        base3[32 * h:32 * h + 32, 72 + 9 * h] = 1.0
    pkb2[:, B3:B3 + 172] = base3
    pkb2[:, WOB:WOB + C] = Wo.T
    esel = np.zeros((36, 9, C), np.float32)
    for o in range(9):
        for hh in range(HEADS):
            esel[9 * hh + o, o, 32 * hh:32 * hh + 32] = 1.0
    pkb2[0:36, E9:E9 + 9 * C] = esel.reshape(36, 9 * C)
    pkb2[64:100, E9:E9 + 9 * C] = esel.reshape(36, 9 * C)
    mt = np.zeros((4, S), np.float32)
    for b in range(B):
        for h in range(HEADS):
            for o in range(9):
                di, dj = o // 3 - 1, o % 3 - 1
                p = 64 * b + 9 * h + o
                mt[0, p] = 1.0 if dj == -1 else 0.0
                mt[1, p] = 1.0 if dj == 1 else 0.0
                mt[2, p] = 1.0 if di == -1 else 0.0
                mt[3, p] = 1.0 if di == 1 else 0.0
    pkb2[0:4, MT:MT + S] = mt

    den_t = np.zeros((S, S), np.float32)
    for b in range(B):
        for h in range(HEADS):
            den_t[64 * b + 9 * h:64 * b + 9 * h + 9,
                  64 * b + 9 * h:64 * b + 9 * h + 9] = 1.0
    pkb2[0:S36, DENB:DENB + S36] = den_t[0:S36, 0:S36]
    pkf = np.zeros((C, NPKF), np.float32)
    pkf[0:S, DEN:DEN + S] = den_t
    pkf[0:S, BA2] = float(ba2[0]) * 0.5   # tanh-sigmoid bias
    pkf[:, BA1] = ba1
    pkf[:, BO] = bo

    in_maps = []
    for c in range(NCORES):
        r0 = 7 * c
        rows = np.clip(np.arange(r0 - 1, r0 + 8), 0, 55)
        cols = np.clip(np.arange(-1, 57), 0, 55)
        xs = x[:, :, rows][:, :, :, cols].transpose(1, 0, 2, 3)
        xflip = x[:, :, r0:r0 + 7, :27:-1].transpose(1, 0, 2, 3)
        pkb1c = pkb1.copy()
        pkb1c[:, XS:XS + 1044] = xs.reshape(C, 1044)
        pkb1c[:, XF:XF + 392] = xflip.reshape(C, 392)
        mv = np.zeros((4, RQ, W), np.float32)
        cc, rr = np.arange(W), r0 + np.arange(RQ)
        mv[0, :, cc == 0] = -BIG
        mv[1, :, cc == 55] = -BIG
        mv[2, rr == 0, :] = -BIG
        mv[3, rr == 55, :] = -BIG
        pkb2c = pkb2.copy()
        pkb2c[0:4, MV:MV + NQ] = mv.reshape(4, NQ)
        in_maps.append({"pkb1": pkb1c, "pkb2": pkb2c, "pkf": pkf})
    return in_maps


_NC = None


def _get_nc():
    global _NC
    if _NC is None:
        _NC = build()
    return _NC


def kernel(**inputs):
    args = {k: np.asarray(v, np.float32) for k, v in inputs.items()}
    nc = _get_nc()
    in_maps = host_inputs(
        args["x"], args["Wq"], args["Wk"], args["Wv"], args["Wo"],
        args["bo"], args["Wa1"], args["ba1"], args["Wa2"], args["ba2"])
    res = run_bass_kernel_spmd(nc, in_maps, core_ids=list(range(NCORES)))
    y = np.empty((B, C, H, W), np.float32)
    for c in range(NCORES):
        y[:, :, 7 * c:7 * c + 7, :] = res.results[c]["out"].transpose(1, 0, 2, 3)
    return y


# revision 3
# speedup vs baseline: 1.0222x; 1.0222x over previous
"""Trainium2 Bass kernel for nn_DiagonalMicroAttention (3x3 neighborhood sparse attention).

Final: PE p-state warmup (junk matmuls bridging the input DMA), priority-ordered
emission for the list scheduler, batch-major pp products on DVE, tanh-based
sigmoid (gelu/tanh + exp LUT sets only, preloaded off the critical path),
asym resize algebra folded to 2 STT + 1 TS on Pool, bf16 softmax tail,
3-way K-evac, rob muls spread DVE-direct/Act-evac/Pool-direct, manually
sequenced PSUM pools so dots/ebp/out banks open as phase-A banks retire.

Sharding: 8 cores x 7 query rows (both batches per core). Channel-major layout.
"""
import numpy as np

import concourse.bass as bass
from concourse.ap import AP
import concourse.tile as tile
from concourse import bacc, mybir
from concourse.bass_utils import run_bass_kernel_spmd

F32 = mybir.dt.float32
BF16 = mybir.dt.bfloat16
AF = mybir.ActivationFunctionType
ALU = mybir.AluOpType

B, C, H, W, HEADS = 2, 128, 56, 56, 4
DH = C // HEADS
SCALE = float(DH) ** -0.5
NCORES = 8
RQ, RH, WP = 7, 9, 58
NQ = RQ * W          # 392 queries per batch per core
BIG = 30000.0
S = 100
S36 = 36
NJUNK = 7

# pkb1 (bf16, 128 x 2240): inputs + projection weights
XS, XF, WQ, WK, WV, WA, WB, W2 = 0, 1044, 1436, 1564, 1692, 1820, 1948, 2076
NPKB1 = 2240
# pkb2 (bf16, 128 x 2044): base3, wo, esel9, mask, den selector
B3, WOB, E9, MT, MV, DENB = 0, 172, 300, 1452, 1552, 1944
NPKB2 = 2044
DEN = 3
BA2, BA1, BO = 0, 1, 2
NPKF = 103

# rob engine per offset o: d=DVE-direct, a=Act-evac+DVE-mul, p=Pool-direct
import os as _os
ROB_ENG = [list(_os.environ.get("ROB0", "dqadqadqa")),
           list(_os.environ.get("ROB1", "ddadqaqda"))]
# pp ops offloaded to Pool: "b0:12,b1:0123" style
_ppp = _os.environ.get("PP_POOL", "")
PP_POOL = set()
for part in _ppp.split(","):
    if ":" in part:
        bs, os_ = part.split(":")
        for ch in os_:
            PP_POOL.add((int(bs[1]), int(ch)))
O_ORDER = list(range(9))


def _emit_body(nc, tc, v):
    work, robpool = v["work"], v["robpool"]
    pkb1, pkb2, pkf, out_d = v["pkb1"], v["pkb2"], v["pkf"], v["out_d"]

    qsb = work.tile([C, B, RQ, W], BF16, tag="qsb")
    ksb = work.tile([C, B, RH, WP], BF16, tag="ksb")
    vsb = work.tile([C, B, RH, WP], BF16, tag="vsb")
    pp = work.tile([C, B, 9, RQ, W], BF16, tag="pp")
    first = "t1_0" not in v
    if first:
        v["a1s"] = work.tile([C, B, RQ, 28], BF16, tag="a1s", name="a1s")
        for b in range(B):
            v[f"apad_{b}"] = work.tile([S36, RQ, 30], BF16, tag=f"apad{b}",
                                       name=f"apad_{b}")
            v[f"u_{b}"] = work.tile([S36, RQ, 28, 2], BF16, tag=f"u{b}",
                                    name=f"u_{b}")
            v[f"t1_{b}"] = work.tile([S36, NQ], BF16, tag=f"t1{b}",
                                     name=f"t1_{b}")
        v["dummy"] = work.tile([1, 1], F32, tag="dummy", name="dummy")
    a1s, dummy = v["a1s"], v["dummy"]
    osb = work.tile([C, B, RQ, W], F32, tag="osb")
    e_t = [work.tile([S36, NQ], BF16, tag=f"e{b}", name=f"e_{b}")
           for b in range(B)]
    r_t = [work.tile([S36, NQ], F32, tag=f"r{b}", name=f"r_{b}")
           for b in range(B)]
    et1 = [work.tile([S36, NQ], BF16, tag=f"et1{b}", name=f"et1_{b}")
           for b in range(B)]
    e2 = [work.tile([S36, NQ], BF16, tag=f"e2{b}", name=f"e2_{b}")
          for b in range(B)]

    xs4 = pkb1[:, XS:XS + 1044].rearrange("p (b r c) -> p b r c", b=B, r=RH)
    xsf = pkb1[:, XS:XS + 1044]
    xf4 = pkb1[:, XF:XF + 392].rearrange("p (b r c) -> p b r c", b=B, r=RQ)
    wob = pkb2[:, WOB:WOB + C]
    ba1_v = pkf[:, BA1:BA1 + 1]
    bo_v = pkf[:, BO:BO + 1]
    ba2h_v = pkf[0:S36, BA2:BA2 + 1]   # host stores ba2*0.5
    mt_v = pkb2[0:4, MT:MT + S36]
    mv_v = pkb2[0:4, MV:MV + NQ]
    w2v = pkb1[:, W2 + 64:W2 + 64 + S36]

    def kshift(t, o):
        di, dj = o // 3 - 1, o % 3 - 1
        return t[:, :, 1 + di:8 + di, 1 + dj:57 + dj]

    NKV = B * RH * WP  # 1044

    # ---- single PSUM pool; bank recycling via per-tag rotation ----
    # tag bankA (2 bufs): qps_b -> dots_b -> den_b
    # tag bankB (4 bufs): K chunks -> V chunks -> ebp tiles
    # tag bankC (2 bufs): a1ps -> a2ps_b -> outp_b
    cmps = tc.tile_pool(name="ps", bufs=1, space="PSUM")
    ps = cmps.__enter__()

    qps = [ps.tile([C, 512], F32, tag="bankA", bufs=2, name=f"qps_{b}")
           for b in range(B)]
    kv = [ps.tile([C, 512], F32, tag="bankB", bufs=4, name=f"kv_{j}")
          for j in range(2)]
    kv.append(ps.tile([C, 20], F32, tag="bankB", bufs=4, name="kv_2"))
    a1ps = ps.tile([C, 392], F32, tag="bankC", bufs=2, name="a1ps")

    if first:
        jw = v["junkw"]
        for i in range(6):
            nc.tensor.matmul(qps[0][0:16, :], jw[:, 0:16], jw[:, :],
                             start=True, stop=True)
        for i in range(3):
            nc.tensor.matmul(qps[0][0:16, 0:128], jw[:, 0:16], jw[:, 0:128],
                             start=True, stop=True)
    # Q projection
    for b in range(B):
        nc.tensor.matmul(qps[b][:, 0:NQ], pkb1[:, WQ:WQ + C],
                         xs4[:, b, 1:8, 1:57], start=True, stop=True)
    nc.scalar.copy(out=qsb[:, 0, :, :], in_=qps[0][:, 0:NQ])
    nc.vector.tensor_copy(out=qsb[:, 1, :, :], in_=qps[1][:, 0:NQ])
    # asym a1 early so gelu->a2->tanh->exp-LUT-load clears Act by ~8us
    skip_asym = v.get("skip_asym")
    if not skip_asym:
        nc.tensor.matmul(a1ps[:, :], pkb1[:, WA:WA + C],
                         xs4[:, :, 1:8, 1:29], start=True, stop=False)
        nc.tensor.matmul(a1ps[:, :], pkb1[:, WB:WB + C],
                         xf4[:, :, :, :], start=False, stop=True)
        nc.scalar.activation(a1s[:, :, :, :], a1ps[:, :], AF.Gelu,
                             bias=ba1_v, scale=1.0)
    # K projection; evacs on DVE so Act is free for the asym chain
    ksf = ksb[:, :, :, :].rearrange("p b r c -> p (b r c)")
    vsf = vsb[:, :, :, :].rearrange("p b r c -> p (b r c)")
    for j, (lo, hi) in enumerate(((0, 512), (512, 1024), (1024, NKV))):
        nc.tensor.matmul(kv[j][:, 0:hi - lo], pkb1[:, WK:WK + C],
                         xsf[:, lo:hi], start=True, stop=True)
    nc.scalar.copy(out=ksf[:, 0:512], in_=kv[0][:, :])
    nc.vector.tensor_copy(out=ksf[:, 512:1024], in_=kv[1][:, :])
    nc.vector.tensor_copy(out=ksf[:, 1024:NKV], in_=kv[2][:, :])
    # V projection (rotates through the same bankB buffers)
    vv_t = [ps.tile([C, 512], F32, tag="bankB", bufs=4, name=f"vv_{j}")
            for j in range(2)]
    vv_t.append(ps.tile([C, 20], F32, tag="bankB", bufs=4, name="vv_2"))
    for j, (lo, hi) in enumerate(((0, 512), (512, 1024), (1024, NKV))):
        nc.tensor.matmul(vv_t[j][:, 0:hi - lo], pkb1[:, WV:WV + C],
                         xsf[:, lo:hi], start=True, stop=True)
    # asym branch (a1/gelu hoisted above, before K)
    if not skip_asym:
        for b in range(B):
            a2ps = ps.tile([S36, RQ * 28], F32, tag="bankC", bufs=2,
                           name=f"a2ps_{b}")
            nc.tensor.matmul(a2ps[:, :], w2v, a1s[:, b, :, :],
                             start=True, stop=True)
            # sigmoid(x) = 0.5 + 0.5*tanh(x/2); the affine is folded into
            # the resize: t1 = (3*tau1 + tau0)/16 + 1.25
            nc.scalar.activation(v[f"apad_{b}"][:, :, 1:29], a2ps[:, :],
                                 AF.Tanh, bias=ba2h_v, scale=0.5)
        # prefetch the Exp LUT set right after the tanhs (ACT idles here)
        v["dummy_inst"] = nc.scalar.activation(
            dummy[:, :], v["apad_1"][0:1, 0:1, 1:2], AF.Exp, scale=1.0)
        for b in range(B):
            apad, u_t = v[f"apad_{b}"], v[f"u_{b}"]
            nc.gpsimd.tensor_copy(out=apad[:, :, 0:1], in_=apad[:, :, 1:2])
            nc.gpsimd.tensor_copy(out=apad[:, :, 29:30],
                                  in_=apad[:, :, 28:29])
            nc.vector.scalar_tensor_tensor(
                out=u_t[:, :, :, 0], in0=apad[:, :, 1:29], scalar=3.0,
                in1=apad[:, :, 0:28], op0=ALU.mult, op1=ALU.add)
            nc.vector.scalar_tensor_tensor(
                out=u_t[:, :, :, 1], in0=apad[:, :, 1:29], scalar=3.0,
                in1=apad[:, :, 2:30], op0=ALU.mult, op1=ALU.add)


    def emit_t1(b):
        if v.get("skip_asym"):
            return
        nc.vector.tensor_scalar(
            out=v[f"t1_{b}"][:, :],
            in0=v[f"u_{b}"][:, :, :, :].rearrange("p r c t -> p (r c t)"),
            scalar1=0.0625, scalar2=1.25, op0=ALU.mult, op1=ALU.add)

    def emit_pp(b):
        for o in O_ORDER:
            if (b, o) in PP_POOL:
                nc.gpsimd.tensor_mul(pp[:, b, o, :, :], qsb[:, b, :, :],
                                     kshift(ksb, o)[:, b])
            else:
                nc.vector.tensor_mul(pp[:, b, o, :, :], qsb[:, b, :, :],
                                     kshift(ksb, o)[:, b])

    dots, den = {}, {}

    def emit_dots(b):
        dots[b] = ps.tile([S36, NQ], F32, tag="bankA", bufs=2, name=f"dots_{b}")
        for i, o in enumerate(O_ORDER):
            s0 = B3 + 72 - o
            nc.tensor.matmul(dots[b][:, :], pkb2[:, s0:s0 + S36],
                             pp[:, b, o, :, :], start=(i == 0), stop=False)
        nc.tensor.matmul(dots[b][:, :], mt_v, mv_v, start=False, stop=True)
        nc.scalar.activation(e_t[b][:, :], dots[b][:, :], AF.Exp, scale=SCALE)

    def emit_tail(b):
        den[b] = ps.tile([S36, NQ], F32, tag="bankA", bufs=2, name=f"den_{b}")
        nc.tensor.matmul(den[b][:, :], pkb2[0:S36, DENB:DENB + S36],
                         e_t[b][:, :], start=True, stop=True)
        nc.vector.reciprocal_approx_fast(r_t[b][:, :], den[b][:, :])
        nc.vector.tensor_mul(et1[b][:, :], e_t[b][:, :], v[f"t1_{b}"][:, :])
        nc.vector.tensor_mul(e2[b][:, :], et1[b][:, :], r_t[b][:, :])

    emit_pp(0)
    emit_t1(0)
    emit_dots(0)
    emit_tail(0)
    nc.scalar.copy(out=vsf[:, 0:512], in_=vv_t[0][:, :])
    nc.scalar.copy(out=vsf[:, 512:1024], in_=vv_t[1][:, :])
    nc.scalar.copy(out=vsf[:, 1024:NKV], in_=vv_t[2][:, :])
    emit_pp(1)
    emit_t1(1)
    emit_dots(1)
    emit_tail(1)

    outp = [ps.tile([C, 512], F32, tag="bankC", bufs=2, name=f"outp_{b}")
            for b in range(B)]

    ebps, robs = {}, {}

    def emit_ebp(b, o):
        ebps[(b, o)] = ps.tile([C, 512], F32, tag="bankB", bufs=4,
                               name=f"ebp_{b}_{o}")
        nc.tensor.matmul(ebps[(b, o)][:, 0:NQ],
                         pkb2[0:S36, E9 + C * o:E9 + C * (o + 1)],
                         e2[b][:, :], start=True, stop=True)

    def emit_rob(b, o):
        vv = kshift(vsb, o)[:, b]
        rob = robpool.tile([C, RQ, W], BF16, tag="rob", name=f"rob_{b}_{o}")
        eng = ROB_ENG[b][o]
        if eng == 'd':
            nc.vector.tensor_mul(rob[:, :, :], ebps[(b, o)][:, 0:NQ], vv)
        else:
            ebsb = work.tile([C, NQ], BF16, tag=f"ebsb{b}_{o % 3}",
                             name=f"ebsb_{b}_{o}")
            nc.scalar.copy(out=ebsb[:, :], in_=ebps[(b, o)][:, 0:NQ])
            if eng == 'q':
                nc.gpsimd.tensor_mul(rob[:, :, :], ebsb[:, :], vv)
            else:
                nc.vector.tensor_mul(rob[:, :, :], ebsb[:, :], vv)
        robs[(b, o)] = rob

    def emit_c(b, split=False):
        for o in range(9):
            emit_ebp(b, o)
            emit_rob(b, o)
        osbf = osb[:, b, :, :].rearrange("p r c -> p (r c)")
        outf = out_d[:, b, :, :].rearrange("p r c -> p (r c)")
        halves = ((0, 196), (196, NQ)) if split else ((0, NQ),)
        for lo, hi in halves:
            for i in range(9):
                nc.tensor.matmul(
                    outp[b][:, lo:hi], wob,
                    robs[(b, i)][:, :, :].rearrange("p r c -> p (r c)")[:, lo:hi],
                    start=(i == 0), stop=(i == 8))
            nc.scalar.activation(osbf[:, lo:hi], outp[b][:, lo:hi],
                                 AF.Identity, bias=bo_v, scale=1.0)
            nc.sync.dma_start(out=outf[:, lo:hi], in_=osbf[:, lo:hi])

    emit_c(0)
    emit_c(1)

    cmps.__exit__(None, None, None)

def build(repeat=1):
    nc = bacc.Bacc(num_devices=NCORES, debug=False)
    pkb1_d = nc.dram_tensor("pkb1", (C, NPKB1), BF16, kind="ExternalInput")
    pkb2_d = nc.dram_tensor("pkb2", (C, NPKB2), BF16, kind="ExternalInput")
    pkf_d = nc.dram_tensor("pkf", (C, NPKF), F32, kind="ExternalInput")
    out_d = nc.dram_tensor("out", (C, B, RQ, W), F32, kind="ExternalOutput")

    with tile.TileContext(nc) as tc:
        with tc.tile_pool(name="consts", bufs=1) as consts, \
             tc.tile_pool(name="work", bufs=1) as work, \
             tc.tile_pool(name="robp", bufs=10) as robpool:
            junkw = consts.tile([C, 512], BF16, tag="junkw")
            dummyg = consts.tile([1, 1], F32, tag="dummyg")
            nc.gpsimd.memset(junkw[:, :], 0.0)
            # preload the Gelu/Tanh LUT set while everything is idle
            nc.scalar.activation(dummyg[:, :], junkw[0:1, 0:1], AF.Gelu,
                                 bias=0.0, scale=1.0)
            pkb1_t = consts.tile([C, NPKB1], BF16, tag="pkb1")
            pkb2_t = consts.tile([C, NPKB2], BF16, tag="pkb2")
            pkf_t = consts.tile([C, NPKF], F32, tag="pkf")
            # x + Wq/Wk/Wv first (gates everything), then pkb2 (dots
            # selectors), then pkf + asym weights via SWDGE
            nc.sync.dma_start(out=pkb1_t[:, 0:WA], in_=pkb1_d[:, 0:WA])
            nc.scalar.dma_start(out=pkb1_t[:, WA:NPKB1],
                                in_=pkb1_d[:, WA:NPKB1])
            nc.scalar.dma_start(out=pkf_t[:, :], in_=pkf_d[:, :])
            nc.scalar.dma_start(out=pkb2_t[:, :], in_=pkb2_d[:, :])
            v = {"pkb1": pkb1_t, "pkb2": pkb2_t, "pkf": pkf_t, "out_d": out_d,
                 "work": work, "robpool": robpool, "junkw": junkw}
            for i in range(repeat):
                v["skip_asym"] = (i > 0)
                _emit_body(nc, tc, v)
    nc.compile()
    return nc


def host_inputs(x, Wq, Wk, Wv, Wo, bo, Wa1, ba1, Wa2, ba2):
    import ml_dtypes
    BF = ml_dtypes.bfloat16
    pkb1 = np.zeros((C, NPKB1), BF)
    pkb1[:, WQ:WQ + C] = Wq.T
    pkb1[:, WK:WK + C] = Wk.T
    pkb1[:, WV:WV + C] = Wv.T
    pkb1[:, WA:WA + C] = Wa1[:, :C].T
    pkb1[:, WB:WB + C] = Wa1[:, C:].T
    w2 = np.zeros((C, 164), np.float32)
    w2[:, 64:100] = Wa2[0][:, None]
    pkb1[:, W2:W2 + 164] = w2

    pkb2 = np.zeros((C, NPKB2), BF)
    base3 = np.zeros((C, 172), np.float32)
    for h in range(HEADS):
        base3[32 * h:32 * h + 32, 72 + 9 * h] = 1.0
    pkb2[:, B3:B3 + 172] = base3
    pkb2[:, WOB:WOB + C] = Wo.T
    esel = np.zeros((36, 9, C), np.float32)
    for o in range(9):
        for hh in range(HEADS):
            esel[9 * hh + o, o, 32 * hh:32 * hh + 32] = 1.0
    pkb2[0:36, E9:E9 + 9 * C] = esel.reshape(36, 9 * C)
    pkb2[64:100, E9:E9 + 9 * C] = esel.reshape(36, 9 * C)
    mt = np.zeros((4, S), np.float32)
    for b in range(B):
        for h in range(HEADS):
            for o in range(9):
                di, dj = o // 3 - 1, o % 3 - 1
                p = 64 * b + 9 * h + o
                mt[0, p] = 1.0 if dj == -1 else 0.0
                mt[1, p] = 1.0 if dj == 1 else 0.0
                mt[2, p] = 1.0 if di == -1 else 0.0
                mt[3, p] = 1.0 if di == 1 else 0.0
    pkb2[0:4, MT:MT + S] = mt

    den_t = np.zeros((S, S), np.float32)
    for b in range(B):
        for h in range(HEADS):
            den_t[64 * b + 9 * h:64 * b + 9 * h + 9,
                  64 * b + 9 * h:64 * b + 9 * h + 9] = 1.0
    pkb2[0:S36, DENB:DENB + S36] = den_t[0:S36, 0:S36]
    pkf = np.zeros((C, NPKF), np.float32)
    pkf[0:S, DEN:DEN + S] = den_t
    pkf[0:S, BA2] = float(ba2[0]) * 0.5   # tanh-sigmoid bias
    pkf[:, BA1] = ba1
    pkf[:, BO] = bo

    in_maps = []
    for c in range(NCORES):
        r0 = 7 * c
        rows = np.clip(np.arange(r0 - 1, r0 + 8), 0, 55)
        cols = np.clip(np.arange(-1, 57), 0, 55)
        xs = x[:, :, rows][:, :, :, cols].transpose(1, 0, 2, 3)
        xflip = x[:, :, r0:r0 + 7, :27:-1].transpose(1, 0, 2, 3)
        pkb1c = pkb1.copy()
        pkb1c[:, XS:XS + 1044] = xs.reshape(C, 1044)
        pkb1c[:, XF:XF + 392] = xflip.reshape(C, 392)
        mv = np.zeros((4, RQ, W), np.float32)
        cc, rr = np.arange(W), r0 + np.arange(RQ)
        mv[0, :, cc == 0] = -BIG
        mv[1, :, cc == 55] = -BIG
        mv[2, rr == 0, :] = -BIG
        mv[3, rr == 55, :] = -BIG
        pkb2c = pkb2.copy()
        pkb2c[0:4, MV:MV + NQ] = mv.reshape(4, NQ)
        in_maps.append({"pkb1": pkb1c, "pkb2": pkb2c, "pkf": pkf})
    return in_maps


_NC = None


def _get_nc():
    global _NC
    if _NC is None:
        _NC = build()
    return _NC


def kernel(**inputs):
    args = {k: np.asarray(v, np.float32) for k, v in inputs.items()}
    nc = _get_nc()
    in_maps = host_inputs(
        args["x"], args["Wq"], args["Wk"], args["Wv"], args["Wo"],
        args["bo"], args["Wa1"], args["ba1"], args["Wa2"], args["ba2"])
    res = run_bass_kernel_spmd(nc, in_maps, core_ids=list(range(NCORES)))
    y = np.empty((B, C, H, W), np.float32)
    for c in range(NCORES):
        y[:, :, 7 * c:7 * c + 7, :] = res.results[c]["out"].transpose(1, 0, 2, 3)
    return y


# revision 4
# speedup vs baseline: 1.0264x; 1.0042x over previous
"""Trainium2 Bass kernel for nn_DiagonalMicroAttention (3x3 neighborhood sparse attention).

Final: PE p-state warmup (junk matmuls bridging the input DMA), priority-ordered
emission for the list scheduler, batch-major pp products on DVE, tanh-based
sigmoid (gelu/tanh + exp LUT sets only, preloaded off the critical path),
asym resize algebra folded to 2 STT + 1 TS on Pool, bf16 softmax tail,
3-way K-evac, rob muls spread DVE-direct/Act-evac/Pool-direct, manually
sequenced PSUM pools so dots/ebp/out banks open as phase-A banks retire.

Sharding: 8 cores x 7 query rows (both batches per core). Channel-major layout.
"""
import numpy as np

import concourse.bass as bass
from concourse.ap import AP
import concourse.tile as tile
from concourse import bacc, mybir
from concourse.bass_utils import run_bass_kernel_spmd

F32 = mybir.dt.float32
BF16 = mybir.dt.bfloat16
AF = mybir.ActivationFunctionType
ALU = mybir.AluOpType

B, C, H, W, HEADS = 2, 128, 56, 56, 4
DH = C // HEADS
SCALE = float(DH) ** -0.5
NCORES = 8
RQ, RH, WP = 7, 9, 58
NQ = RQ * W          # 392 queries per batch per core
BIG = 30000.0
S = 100
S36 = 36
NJUNK = 7

# pkb1 (bf16, 128 x 2240): inputs + projection weights
XS, XF, WQ, WK, WV, WA, WB, W2 = 0, 1044, 1436, 1564, 1692, 1820, 1948, 2076
NPKB1 = 2240
# pkb2 (bf16, 128 x 2044): base3, wo, esel9, mask, den selector
B3, WOB, E9, MT, MV, DENB = 0, 172, 300, 1452, 1552, 1944
NPKB2 = 2044
DEN = 3
BA2, BA1, BO = 0, 1, 2
NPKF = 103

# rob engine per offset o: d=DVE-direct, a=Act-evac+DVE-mul, p=Pool-direct
import os as _os
ROB_ENG = [list(_os.environ.get("ROB0", "ddadqaqda")),
           list(_os.environ.get("ROB1", "ddadqaqda"))]
# pp ops offloaded to Pool: "b0:12,b1:0123" style
_ppp = _os.environ.get("PP_POOL", "")
PP_POOL = set()
for part in _ppp.split(","):
    if ":" in part:
        bs, os_ = part.split(":")
        for ch in os_:
            PP_POOL.add((int(bs[1]), int(ch)))
O_ORDER = list(range(9))


def _emit_body(nc, tc, v):
    work, robpool = v["work"], v["robpool"]
    pkb1, pkb2, pkf, out_d = v["pkb1"], v["pkb2"], v["pkf"], v["out_d"]

    qsb = work.tile([C, B, RQ, W], BF16, tag="qsb")
    ksb = work.tile([C, B, RH, WP], BF16, tag="ksb")
    vsb = work.tile([C, B, RH, WP], BF16, tag="vsb")
    pp = work.tile([C, B, 9, RQ, W], BF16, tag="pp")
    first = "t1_0" not in v
    if first:
        v["a1s"] = work.tile([C, B, RQ, 28], BF16, tag="a1s", name="a1s")
        for b in range(B):
            v[f"apad_{b}"] = work.tile([S36, RQ, 30], BF16, tag=f"apad{b}",
                                       name=f"apad_{b}")
            v[f"u_{b}"] = work.tile([S36, RQ, 28, 2], BF16, tag=f"u{b}",
                                    name=f"u_{b}")
            v[f"t1_{b}"] = work.tile([S36, NQ], BF16, tag=f"t1{b}",
                                     name=f"t1_{b}")
        v["dummy"] = work.tile([1, 1], F32, tag="dummy", name="dummy")
    a1s, dummy = v["a1s"], v["dummy"]
    osb = work.tile([C, B, RQ, W], F32, tag="osb")
    e_t = [work.tile([S36, NQ], BF16, tag=f"e{b}", name=f"e_{b}")
           for b in range(B)]
    r_t = [work.tile([S36, NQ], F32, tag=f"r{b}", name=f"r_{b}")
           for b in range(B)]
    et1 = [work.tile([S36, NQ], BF16, tag=f"et1{b}", name=f"et1_{b}")
           for b in range(B)]
    e2 = [work.tile([S36, NQ], BF16, tag=f"e2{b}", name=f"e2_{b}")
          for b in range(B)]

    xs4 = pkb1[:, XS:XS + 1044].rearrange("p (b r c) -> p b r c", b=B, r=RH)
    xsf = pkb1[:, XS:XS + 1044]
    xf4 = pkb1[:, XF:XF + 392].rearrange("p (b r c) -> p b r c", b=B, r=RQ)
    wob = pkb2[:, WOB:WOB + C]
    ba1_v = pkf[:, BA1:BA1 + 1]
    bo_v = pkf[:, BO:BO + 1]
    ba2h_v = pkf[0:S36, BA2:BA2 + 1]   # host stores ba2*0.5
    mt_v = pkb2[0:4, MT:MT + S36]
    mv_v = pkb2[0:4, MV:MV + NQ]
    w2v = pkb1[:, W2 + 64:W2 + 64 + S36]

    def kshift(t, o):
        di, dj = o // 3 - 1, o % 3 - 1
        return t[:, :, 1 + di:8 + di, 1 + dj:57 + dj]

    NKV = B * RH * WP  # 1044

    # ---- single PSUM pool; bank recycling via per-tag rotation ----
    # tag bankA (2 bufs): qps_b -> dots_b -> den_b
    # tag bankB (4 bufs): K chunks -> V chunks -> ebp tiles
    # tag bankC (2 bufs): a1ps -> a2ps_b -> outp_b
    cmps = tc.tile_pool(name="ps", bufs=1, space="PSUM")
    ps = cmps.__enter__()

    qps = [ps.tile([C, 512], F32, tag="bankA", bufs=2, name=f"qps_{b}")
           for b in range(B)]
    kv = [ps.tile([C, 512], F32, tag="bankB", bufs=4, name=f"kv_{j}")
          for j in range(2)]
    kv.append(ps.tile([C, 20], F32, tag="bankB", bufs=4, name="kv_2"))
    a1ps = ps.tile([C, 392], F32, tag="bankC", bufs=2, name="a1ps")

    if first:
        jw = v["junkw"]
        for i in range(6):
            nc.tensor.matmul(qps[0][0:16, :], jw[:, 0:16], jw[:, :],
                             start=True, stop=True)
        for i in range(3):
            nc.tensor.matmul(qps[0][0:16, 0:128], jw[:, 0:16], jw[:, 0:128],
                             start=True, stop=True)
    # Q projection
    for b in range(B):
        nc.tensor.matmul(qps[b][:, 0:NQ], pkb1[:, WQ:WQ + C],
                         xs4[:, b, 1:8, 1:57], start=True, stop=True)
    nc.scalar.copy(out=qsb[:, 0, :, :], in_=qps[0][:, 0:NQ])
    nc.vector.tensor_copy(out=qsb[:, 1, :, :], in_=qps[1][:, 0:NQ])
    # asym a1 early so gelu->a2->tanh->exp-LUT-load clears Act by ~8us
    skip_asym = v.get("skip_asym")
    if not skip_asym:
        nc.tensor.matmul(a1ps[:, :], pkb1[:, WA:WA + C],
                         xs4[:, :, 1:8, 1:29], start=True, stop=False)
        nc.tensor.matmul(a1ps[:, :], pkb1[:, WB:WB + C],
                         xf4[:, :, :, :], start=False, stop=True)
        nc.scalar.activation(a1s[:, :, :, :], a1ps[:, :], AF.Gelu,
                             bias=ba1_v, scale=1.0)
    # K projection; evacs on DVE so Act is free for the asym chain
    ksf = ksb[:, :, :, :].rearrange("p b r c -> p (b r c)")
    vsf = vsb[:, :, :, :].rearrange("p b r c -> p (b r c)")
    for j, (lo, hi) in enumerate(((0, 512), (512, 1024), (1024, NKV))):
        nc.tensor.matmul(kv[j][:, 0:hi - lo], pkb1[:, WK:WK + C],
                         xsf[:, lo:hi], start=True, stop=True)
    nc.scalar.copy(out=ksf[:, 0:512], in_=kv[0][:, :])
    nc.vector.tensor_copy(out=ksf[:, 512:1024], in_=kv[1][:, :])
    nc.vector.tensor_copy(out=ksf[:, 1024:NKV], in_=kv[2][:, :])
    # V projection (rotates through the same bankB buffers)
    vv_t = [ps.tile([C, 512], F32, tag="bankB", bufs=4, name=f"vv_{j}")
            for j in range(2)]
    vv_t.append(ps.tile([C, 20], F32, tag="bankB", bufs=4, name="vv_2"))
    for j, (lo, hi) in enumerate(((0, 512), (512, 1024), (1024, NKV))):
        nc.tensor.matmul(vv_t[j][:, 0:hi - lo], pkb1[:, WV:WV + C],
                         xsf[:, lo:hi], start=True, stop=True)
    # asym branch (a1/gelu hoisted above, before K)
    if not skip_asym:
        for b in range(B):
            a2ps = ps.tile([S36, RQ * 28], F32, tag="bankC", bufs=2,
                           name=f"a2ps_{b}")
            nc.tensor.matmul(a2ps[:, :], w2v, a1s[:, b, :, :],
                             start=True, stop=True)
            # sigmoid(x) = 0.5 + 0.5*tanh(x/2); the affine is folded into
            # the resize: t1 = (3*tau1 + tau0)/16 + 1.25
            nc.scalar.activation(v[f"apad_{b}"][:, :, 1:29], a2ps[:, :],
                                 AF.Tanh, bias=ba2h_v, scale=0.5)
        # prefetch the Exp LUT set right after the tanhs (ACT idles here)
        v["dummy_inst"] = nc.scalar.activation(
            dummy[:, :], v["apad_1"][0:1, 0:1, 1:2], AF.Exp, scale=1.0)
        for b in range(B):
            apad, u_t = v[f"apad_{b}"], v[f"u_{b}"]
            nc.gpsimd.tensor_copy(out=apad[:, :, 0:1], in_=apad[:, :, 1:2])
            nc.gpsimd.tensor_copy(out=apad[:, :, 29:30],
                                  in_=apad[:, :, 28:29])
            nc.vector.scalar_tensor_tensor(
                out=u_t[:, :, :, 0], in0=apad[:, :, 1:29], scalar=3.0,
                in1=apad[:, :, 0:28], op0=ALU.mult, op1=ALU.add)
            nc.vector.scalar_tensor_tensor(
                out=u_t[:, :, :, 1], in0=apad[:, :, 1:29], scalar=3.0,
                in1=apad[:, :, 2:30], op0=ALU.mult, op1=ALU.add)


    def emit_t1(b):
        if v.get("skip_asym"):
            return
        nc.vector.tensor_scalar(
            out=v[f"t1_{b}"][:, :],
            in0=v[f"u_{b}"][:, :, :, :].rearrange("p r c t -> p (r c t)"),
            scalar1=0.0625, scalar2=1.25, op0=ALU.mult, op1=ALU.add)

    def emit_pp(b):
        for o in O_ORDER:
            if (b, o) in PP_POOL:
                nc.gpsimd.tensor_mul(pp[:, b, o, :, :], qsb[:, b, :, :],
                                     kshift(ksb, o)[:, b])
            else:
                nc.vector.tensor_mul(pp[:, b, o, :, :], qsb[:, b, :, :],
                                     kshift(ksb, o)[:, b])

    dots, den = {}, {}

    def emit_dots(b):
        dots[b] = ps.tile([S36, NQ], F32, tag="bankA", bufs=2, name=f"dots_{b}")
        for i, o in enumerate(O_ORDER):
            s0 = B3 + 72 - o
            nc.tensor.matmul(dots[b][:, :], pkb2[:, s0:s0 + S36],
                             pp[:, b, o, :, :], start=(i == 0), stop=False)
        nc.tensor.matmul(dots[b][:, :], mt_v, mv_v, start=False, stop=True)
        nc.scalar.activation(e_t[b][:, :], dots[b][:, :], AF.Exp, scale=SCALE)

    def emit_tail(b):
        den[b] = ps.tile([S36, NQ], F32, tag="bankA", bufs=2, name=f"den_{b}")
        nc.tensor.matmul(den[b][:, :], pkb2[0:S36, DENB:DENB + S36],
                         e_t[b][:, :], start=True, stop=True)
        nc.vector.reciprocal_approx_fast(r_t[b][:, :], den[b][:, :])
        nc.vector.tensor_mul(et1[b][:, :], e_t[b][:, :], v[f"t1_{b}"][:, :])
        nc.vector.tensor_mul(e2[b][:, :], et1[b][:, :], r_t[b][:, :])

    emit_pp(0)
    emit_t1(0)
    emit_dots(0)
    emit_tail(0)
    nc.scalar.copy(out=vsf[:, 0:512], in_=vv_t[0][:, :])
    nc.scalar.copy(out=vsf[:, 512:1024], in_=vv_t[1][:, :])
    nc.scalar.copy(out=vsf[:, 1024:NKV], in_=vv_t[2][:, :])
    emit_pp(1)
    emit_t1(1)
    emit_dots(1)
    emit_tail(1)

    outp = [ps.tile([C, 512], F32, tag="bankC", bufs=2, name=f"outp_{b}")
            for b in range(B)]

    ebps, robs = {}, {}

    def emit_ebp(b, o):
        ebps[(b, o)] = ps.tile([C, 512], F32, tag="bankB", bufs=4,
                               name=f"ebp_{b}_{o}")
        nc.tensor.matmul(ebps[(b, o)][:, 0:NQ],
                         pkb2[0:S36, E9 + C * o:E9 + C * (o + 1)],
                         e2[b][:, :], start=True, stop=True)

    def emit_rob(b, o):
        vv = kshift(vsb, o)[:, b]
        rob = robpool.tile([C, RQ, W], BF16, tag="rob", name=f"rob_{b}_{o}")
        eng = ROB_ENG[b][o]
        if eng == 'd':
            nc.vector.tensor_mul(rob[:, :, :], ebps[(b, o)][:, 0:NQ], vv)
        else:
            ebsb = work.tile([C, NQ], BF16, tag=f"ebsb{b}_{o % 3}",
                             name=f"ebsb_{b}_{o}")
            nc.scalar.copy(out=ebsb[:, :], in_=ebps[(b, o)][:, 0:NQ])
            if eng == 'q':
                nc.gpsimd.tensor_mul(rob[:, :, :], ebsb[:, :], vv)
            else:
                nc.vector.tensor_mul(rob[:, :, :], ebsb[:, :], vv)
        robs[(b, o)] = rob

    def emit_c(b, split=False):
        for o in range(9):
            emit_ebp(b, o)
            emit_rob(b, o)
        osbf = osb[:, b, :, :].rearrange("p r c -> p (r c)")
        outf = out_d[:, b, :, :].rearrange("p r c -> p (r c)")
        halves = ((0, 196), (196, NQ)) if split else ((0, NQ),)
        for lo, hi in halves:
            for i in range(9):
                nc.tensor.matmul(
                    outp[b][:, lo:hi], wob,
                    robs[(b, i)][:, :, :].rearrange("p r c -> p (r c)")[:, lo:hi],
                    start=(i == 0), stop=(i == 8))
            nc.scalar.activation(osbf[:, lo:hi], outp[b][:, lo:hi],
                                 AF.Identity, bias=bo_v, scale=1.0)
            nc.sync.dma_start(out=outf[:, lo:hi], in_=osbf[:, lo:hi])

    emit_c(0)
    emit_c(1)

    cmps.__exit__(None, None, None)

def build(repeat=1):
    nc = bacc.Bacc(num_devices=NCORES, debug=False)
    pkb1_d = nc.dram_tensor("pkb1", (C, NPKB1), BF16, kind="ExternalInput")
    pkb2_d = nc.dram_tensor("pkb2", (C, NPKB2), BF16, kind="ExternalInput")
    pkf_d = nc.dram_tensor("pkf", (C, NPKF), F32, kind="ExternalInput")
    out_d = nc.dram_tensor("out", (C, B, RQ, W), F32, kind="ExternalOutput")

    with tile.TileContext(nc) as tc:
        with tc.tile_pool(name="consts", bufs=1) as consts, \
             tc.tile_pool(name="work", bufs=1) as work, \
             tc.tile_pool(name="robp", bufs=10) as robpool:
            junkw = consts.tile([C, 512], BF16, tag="junkw")
            dummyg = consts.tile([1, 1], F32, tag="dummyg")
            nc.gpsimd.memset(junkw[:, :], 0.0)
            # preload the Gelu/Tanh LUT set while everything is idle
            nc.scalar.activation(dummyg[:, :], junkw[0:1, 0:1], AF.Gelu,
                                 bias=0.0, scale=1.0)
            pkb1_t = consts.tile([C, NPKB1], BF16, tag="pkb1")
            pkb2_t = consts.tile([C, NPKB2], BF16, tag="pkb2")
            pkf_t = consts.tile([C, NPKF], F32, tag="pkf")
            # x + Wq/Wk/Wv first (gates everything), then pkb2 (dots
            # selectors), then pkf + asym weights via SWDGE
            nc.sync.dma_start(out=pkb1_t[:, 0:WA], in_=pkb1_d[:, 0:WA])
            nc.scalar.dma_start(out=pkb1_t[:, WA:NPKB1],
                                in_=pkb1_d[:, WA:NPKB1])
            nc.scalar.dma_start(out=pkf_t[:, :], in_=pkf_d[:, :])
            nc.scalar.dma_start(out=pkb2_t[:, :], in_=pkb2_d[:, :])
            v = {"pkb1": pkb1_t, "pkb2": pkb2_t, "pkf": pkf_t, "out_d": out_d,
                 "work": work, "robpool": robpool, "junkw": junkw}
            for i in range(repeat):
                v["skip_asym"] = (i > 0)
                _emit_body(nc, tc, v)
    nc.compile()
    return nc


def host_inputs(x, Wq, Wk, Wv, Wo, bo, Wa1, ba1, Wa2, ba2):
    import ml_dtypes
    BF = ml_dtypes.bfloat16
    pkb1 = np.zeros((C, NPKB1), BF)
    pkb1[:, WQ:WQ + C] = Wq.T
    pkb1[:, WK:WK + C] = Wk.T
    pkb1[:, WV:WV + C] = Wv.T
    pkb1[:, WA:WA + C] = Wa1[:, :C].T
    pkb1[:, WB:WB + C] = Wa1[:, C:].T
    w2 = np.zeros((C, 164), np.float32)
    w2[:, 64:100] = Wa2[0][:, None]
    pkb1[:, W2:W2 + 164] = w2

    pkb2 = np.zeros((C, NPKB2), BF)
    base3 = np.zeros((C, 172), np.float32)
    for h in range(HEADS):
        base3[32 * h:32 * h + 32, 72 + 9 * h] = 1.0
    pkb2[:, B3:B3 + 172] = base3
    pkb2[:, WOB:WOB + C] = Wo.T
    esel = np.zeros((36, 9, C), np.float32)
    for o in range(9):
        for hh in range(HEADS):
            esel[9 * hh + o, o, 32 * hh:32 * hh + 32] = 1.0
    pkb2[0:36, E9:E9 + 9 * C] = esel.reshape(36, 9 * C)
    pkb2[64:100, E9:E9 + 9 * C] = esel.reshape(36, 9 * C)
    mt = np.zeros((4, S), np.float32)
    for b in range(B):
        for h in range(HEADS):
            for o in range(9):
                di, dj = o // 3 - 1, o % 3 - 1
                p = 64 * b + 9 * h + o
                mt[0, p] = 1.0 if dj == -1 else 0.0
                mt[1, p] = 1.0 if dj == 1 else 0.0
                mt[2, p] = 1.0 if di == -1 else 0.0
                mt[3, p] = 1.0 if di == 1 else 0.0
    pkb2[0:4, MT:MT + S] = mt

    den_t = np.zeros((S, S), np.float32)
    for b in range(B):
        for h in range(HEADS):
            den_t[64 * b + 9 * h:64 * b + 9 * h + 9,
                  64 * b + 9 * h:64 * b + 9 * h + 9] = 1.0
    pkb2[0:S36, DENB:DENB + S36] = den_t[0:S36, 0:S36]
    pkf = np.zeros((C, NPKF), np.float32)
    pkf[0:S, DEN:DEN + S] = den_t
    pkf[0:S, BA2] = float(ba2[0]) * 0.5   # tanh-sigmoid bias
    pkf[:, BA1] = ba1
    pkf[:, BO] = bo

    in_maps = []
    for c in range(NCORES):
        r0 = 7 * c
        rows = np.clip(np.arange(r0 - 1, r0 + 8), 0, 55)
        cols = np.clip(np.arange(-1, 57), 0, 55)
        xs = x[:, :, rows][:, :, :, cols].transpose(1, 0, 2, 3)
        xflip = x[:, :, r0:r0 + 7, :27:-1].transpose(1, 0, 2, 3)
        pkb1c = pkb1.copy()
        pkb1c[:, XS:XS + 1044] = xs.reshape(C, 1044)
        pkb1c[:, XF:XF + 392] = xflip.reshape(C, 392)
        mv = np.zeros((4, RQ, W), np.float32)
        cc, rr = np.arange(W), r0 + np.arange(RQ)
        mv[0, :, cc == 0] = -BIG
        mv[1, :, cc == 55] = -BIG
        mv[2, rr == 0, :] = -BIG
        mv[3, rr == 55, :] = -BIG
        pkb2c = pkb2.copy()
        pkb2c[0:4, MV:MV + NQ] = mv.reshape(4, NQ)
        in_maps.append({"pkb1": pkb1c, "pkb2": pkb2c, "pkf": pkf})
    return in_maps


_NC = None


def _get_nc():
    global _NC
    if _NC is None:
        _NC = build()
    return _NC


def kernel(**inputs):
    args = {k: np.asarray(v, np.float32) for k, v in inputs.items()}
    nc = _get_nc()
    in_maps = host_inputs(
        args["x"], args["Wq"], args["Wk"], args["Wv"], args["Wo"],
        args["bo"], args["Wa1"], args["ba1"], args["Wa2"], args["ba2"])
    res = run_bass_kernel_spmd(nc, in_maps, core_ids=list(range(NCORES)))
    y = np.empty((B, C, H, W), np.float32)
    for c in range(NCORES):
        y[:, :, 7 * c:7 * c + 7, :] = res.results[c]["out"].transpose(1, 0, 2, 3)
    return y
